# revision 2
# baseline (speedup 1.0000x reference)
"""Trainium2 Bass kernel for additive-attention pooling.

reference math:
    scores[b,t] = tanh(q[b]) @ vw_a + tanh(c[b,t]) @ vw_b
    attn        = softmax(where(mask<1, -1e10, scores), axis=t)
    out[b,e]    = sum_t attn[b,t] * c[b,t,e]

Softmax is shift-invariant and the query term is constant over t, so the
output does not depend on `query` or `v_w[:E]` at all.  Per batch row we
compute, in a single pass over context:
    s_t  = tanh(c_t) . w2                (ACT tanh + DVE fused mul-reduce)
    p_t  = exp(s_t + (mask_t - 1)*1e9)   (ACT exp with per-partition bias;
                                          masked lanes underflow to exactly 0)
    out  = (sum_t p_t * c_t) / (sum_t p_t)   (PE matmuls accumulating in PSUM)

Sharding: pure data parallel, batch 16 -> 2 per core on 8 cores; w2
replicated.  No collectives needed.
"""

import sys

for _p in ("/opt/trn_rl_repo", "/root/.axon_site/_ro/trn_rl_repo"):
    if _p not in sys.path:
        sys.path.append(_p)

import numpy as np

B, T, E = 16, 4096, 768
NCORES = 8
BPC = B // NCORES  # batches per core
P = 128            # partitions per tile
G = T // P         # 32 t-tiles per batch
NEG_BIG = 1.0e9    # exp(-1e9) == 0.0 in f32

_cache = {}


def _build_program():
    import concourse.tile as tile
    from concourse import bacc, mybir

    f32 = mybir.dt.float32
    i32 = mybir.dt.int32
    AF = mybir.ActivationFunctionType
    ALU = mybir.AluOpType

    nc = bacc.Bacc(
        "TRN2",
        target_bir_lowering=False,
        debug=False,
        enable_asserts=False,
        num_devices=NCORES,
    )
    ctx_d = nc.dram_tensor("ctx", [BPC, T, E], f32, kind="ExternalInput")
    mask_d = nc.dram_tensor("mask", [BPC, T], i32, kind="ExternalInput")
    w2_d = nc.dram_tensor("w2", [1, E], f32, kind="ExternalInput")
    out_d = nc.dram_tensor("out", [BPC, E], f32, kind="ExternalOutput")

    with tile.TileContext(nc) as tc:
        with (
            tc.tile_pool(name="const", bufs=1) as const_pool,
            tc.tile_pool(name="cin", bufs=6) as c_pool,
            tc.tile_pool(name="tanh", bufs=3) as t_pool,
            tc.tile_pool(name="small", bufs=6) as s_pool,
            tc.tile_pool(name="batch", bufs=2) as b_pool,
            tc.tile_pool(name="paccum", bufs=2, space="PSUM") as pa_pool,
            tc.tile_pool(name="pden", bufs=2, space="PSUM") as pd_pool,
        ):
            # w2 replicated across all 128 partitions (constant for the
            # whole kernel), plus a ones column for the denominator matmul.
            w2_rep = const_pool.tile([P, E], f32)
            nc.sync.dma_start(w2_rep[0:1, :], w2_d[:])
            nc.gpsimd.partition_broadcast(w2_rep[:, :], w2_rep[0:1, :])
            ones = const_pool.tile([P, 1], f32)
            nc.gpsimd.memset(ones[:], 1.0)

            for b in range(BPC):
                # mask -> additive bias: 0 where kept, -1e9 where masked.
                mask_i = b_pool.tile([P, G], i32)
                nc.sync.dma_start(
                    mask_i[:], mask_d[b].rearrange("(g p) -> p g", p=P)
                )
                mask_f = b_pool.tile([P, G], f32)
                nc.vector.tensor_copy(mask_f[:], mask_i[:])
                mask_bias = b_pool.tile([P, G], f32)
                nc.vector.tensor_scalar(
                    mask_bias[:], mask_f[:], NEG_BIG, NEG_BIG,
                    op0=ALU.mult, op1=ALU.subtract,
                )

                acc = pa_pool.tile([1, E], f32)   # sum_t p_t * c_t
                den = pd_pool.tile([1, 1], f32)   # sum_t p_t

                for g in range(G):
                    c = c_pool.tile([P, E], f32)
                    nc.sync.dma_start(c[:], ctx_d[b, g * P:(g + 1) * P, :])

                    th = t_pool.tile([P, E], f32)
                    nc.scalar.activation(th[:], c[:], AF.Tanh)

                    # s = sum_e tanh(c)*w2 (fused multiply+reduce on DVE;
                    # the elementwise product overwrites th in place)
                    s_col = s_pool.tile([P, 1], f32)
                    nc.vector.affine_mul_reduce(
                        th[:], s_col[:], th[:], w2_rep[:], 1.0, 0.0
                    )

                    # p = exp(s + mask_bias)
                    p_col = s_pool.tile([P, 1], f32)
                    nc.scalar.activation(
                        p_col[:], s_col[:], AF.Exp,
                        bias=mask_bias[:, g:g + 1],
                    )

                    first, last = g == 0, g == G - 1
                    nc.tensor.matmul(
                        acc[:, 0:512], lhsT=p_col[:], rhs=c[:, 0:512],
                        start=first, stop=last,
                    )
                    nc.tensor.matmul(
                        acc[:, 512:E], lhsT=p_col[:], rhs=c[:, 512:E],
                        start=first, stop=last,
                    )
                    nc.tensor.matmul(
                        den[:], lhsT=p_col[:], rhs=ones[:],
                        start=first, stop=last,
                    )

                recip = s_pool.tile([1, 1], f32)
                nc.vector.reciprocal(recip[:], den[:])
                out_sb = s_pool.tile([1, E], f32)
                nc.vector.tensor_scalar_mul(out_sb[:], acc[:], recip[:])
                nc.sync.dma_start(out_d[b:b + 1, :], out_sb[:])

    nc.compile()
    return nc


def _get_program():
    if "nc" not in _cache:
        _cache["nc"] = _build_program()
    return _cache["nc"]


def kernel(query, context, mask, v_w):
    from concourse.bass_utils import run_bass_kernel_spmd

    nc = _get_program()
    w2 = np.ascontiguousarray(v_w[E:]).reshape(1, E).astype(np.float32)
    in_maps = [
        {
            "ctx": np.ascontiguousarray(context[i * BPC:(i + 1) * BPC]),
            "mask": np.ascontiguousarray(mask[i * BPC:(i + 1) * BPC]),
            "w2": w2,
        }
        for i in range(NCORES)
    ]
    res = run_bass_kernel_spmd(nc, in_maps, list(range(NCORES)))
    return np.concatenate([res.results[i]["out"] for i in range(NCORES)], axis=0)
